# revision 13
# baseline (speedup 1.0000x reference)
"""Distributed causal multi-head attention (B=2, S=2048, H=2048, 16 heads) on
8 TRN2 NeuronCores.

Sharding: core c handles batch b = c // 4 and the 4-head group g = c % 4
(heads 4g..4g+3). Each core computes its heads' QKV projection, causal
attention, and the partial output projection against its 512 columns of
w_proj (Megatron row-parallel). No on-device collectives: the 4 partial
outputs per batch are summed on the host during unsharding.

Device compute is bf16 on the TensorEngine with fp32 PSUM accumulation;
softmax runs in fp32. Scores are bounded (~±5 post-scale) for this data
distribution, so exp needs no max-subtraction (softmax is shift-invariant,
no overflow risk). Attention scores are computed directly in transposed
orientation (scoresT[k_pos, q_pos] via lhsT=k-tile, rhs=q-block), which is
what the PV matmul wants as its moving operand — no on-chip transposes at
all. The softmax denominator comes from a ones matmul over probsT (column
sums, replicated to all 128 partitions so the reciprocal runs wide), and
1/l is applied when draining the PV accumulator. The v-projection is
interleaved into the attention loop so attention starts right after the
q/k projections.
"""
import sys

for _p in ("/opt/trn_rl_repo", "/opt/pypackages"):
    if _p not in sys.path:
        sys.path.append(_p)

import numpy as np
from ml_dtypes import bfloat16

import concourse.bass as bass  # noqa: F401
import concourse.mybir as mybir
import concourse.tile as tile
import concourse.bacc as bacc
from concourse.bass_utils import run_bass_kernel_spmd

B, S, H = 2, 2048, 2048
NH = 16
HD = 128
NHL = 4                      # heads per core
HL = NHL * HD                # 512 local head-dims
N_CORES = 8
F32 = mybir.dt.float32
BF16 = mybir.dt.bfloat16
SCALE = float(1.0 / np.sqrt(HD))
NEG = -679.0                 # additive pre-scale mask; ~ -60 after SCALE

NHT = H // 128               # 16 contraction tiles
NQ = S // 128                # 16 query sub-tiles
NBLK = 4                     # 512-wide query blocks

LAST_RESULT = None
_CACHED_NC = None


def build_graph():
    nc = bacc.Bacc("TRN2", target_bir_lowering=False, num_devices=N_CORES)
    xT_d = nc.declare_dram_parameter("xT", [H, S], BF16, isOutput=False)
    wqT_d = nc.declare_dram_parameter("wqT", [H, HL], BF16, isOutput=False)
    wkT_d = nc.declare_dram_parameter("wkT", [H, HL], BF16, isOutput=False)
    wvT_d = nc.declare_dram_parameter("wvT", [H, HL], BF16, isOutput=False)
    wpT_d = nc.declare_dram_parameter("wpT", [HL, H], BF16, isOutput=False)
    # strict-lower-triangular NEG in [k_pos, q_pos] orientation
    mask_d = nc.declare_dram_parameter("mask", [128, 128], F32, isOutput=False)
    onesb_d = nc.declare_dram_parameter("onesb", [128, 128], BF16, isOutput=False)
    out_d = nc.declare_dram_parameter("out", [S, H], F32, isOutput=True)

    with tile.TileContext(nc) as tc:
        with tc.tile_pool(name="persist", bufs=1) as pp:
            mask_sb = pp.tile([128, 128], F32, tag="mask", name="mask_sb")
            nc.sync.dma_start(mask_sb[:], mask_d[:])
            onesb_sb = pp.tile([128, 128], BF16, tag="onesb", name="onesb_sb")
            nc.sync.dma_start(onesb_sb[:], onesb_d[:])
            wp_sb = [pp.tile([128, H], BF16, tag=f"wp{k}", name=f"wp{k}")
                     for k in range(4)]
            qT_sb = [pp.tile([128, S], BF16, tag=f"q{h}", name=f"qT{h}")
                     for h in range(NHL)]
            kT_sb = [pp.tile([128, S], BF16, tag=f"k{h}", name=f"kT{h}")
                     for h in range(NHL)]
            v_sb = [pp.tile([128, HL], BF16, tag=f"v{t}", name=f"v{t}")
                    for t in range(NQ)]
            wv_sb = [pp.tile([128, HL], BF16, tag=f"wv{h}", name=f"wvs{h}")
                     for h in range(NHT)]
            yT_sb = [[pp.tile([128, 512], BF16, tag=f"y{h}_{j}", name=f"y{h}_{j}")
                      for j in range(NBLK)] for h in range(NHL)]
            xT_sb = [pp.tile([128, S], BF16, tag=f"x{h}", name=f"x{h}")
                     for h in range(NHT)]
            x_loaded = [False] * NHT

            def load_x(h):
                if not x_loaded[h]:
                    nc.sync.dma_start(
                        xT_sb[h][:], xT_d[128 * h:128 * (h + 1), :])
                    x_loaded[h] = True

            # ---------------- Phase 1: Q/K projections ----------------
            with (
                tc.tile_pool(name="wstream", bufs=6) as wsp,
                tc.tile_pool(name="pmm", bufs=8, space="PSUM") as pmm,
            ):
                def proj_qk(wdram, dst):
                    for head in range(NHL):
                        pss = [pmm.tile([128, 512], F32, tag="mm", name="mm")
                               for _ in range(4)]
                        for h in range(NHT):
                            load_x(h)
                            wt = wsp.tile([128, 128], BF16, tag="w", name="w")
                            nc.sync.dma_start(
                                wt[:],
                                wdram[128 * h:128 * (h + 1),
                                      128 * head:128 * (head + 1)])
                            for s in range(4):
                                nc.tensor.matmul(
                                    pss[s][:], wt[:],
                                    xT_sb[h][:, 512 * s:512 * (s + 1)],
                                    start=(h == 0), stop=(h == NHT - 1))
                        for s in range(4):
                            nc.vector.tensor_copy(
                                dst[head][:, 512 * s:512 * (s + 1)], pss[s][:])

                proj_qk(wqT_d, qT_sb)
                for k in range(4):
                    nc.sync.dma_start(wp_sb[k][:],
                                      wpT_d[128 * k:128 * (k + 1), :])
                proj_qk(wkT_d, kT_sb)
                for h in range(NHT):
                    nc.sync.dma_start(wv_sb[h][:],
                                      wvT_d[128 * h:128 * (h + 1), :])

            # ---------- Phase 2: v-projection + causal attention ----------
            with (
                tc.tile_pool(name="ptp", bufs=4) as ptp,
                tc.tile_pool(name="stat", bufs=6) as stp,
                tc.tile_pool(name="psc", bufs=4, space="PSUM") as psc,
                tc.tile_pool(name="pyt", bufs=2, space="PSUM") as pyt,
                tc.tile_pool(name="pls", bufs=1, space="PSUM") as pls,
            ):
                for j in range(NBLK):
                    for st in range(4 * j, 4 * j + 4):
                        ps = psc.tile([128, 512], F32, tag="sc", name="vmm")
                        for h2 in range(NHT):
                            nc.tensor.matmul(
                                ps[:], xT_sb[h2][:, 128 * st:128 * (st + 1)],
                                wv_sb[h2][:], start=(h2 == 0),
                                stop=(h2 == NHT - 1))
                        nc.vector.tensor_copy(v_sb[st][:], ps[:])
                    for h in range(NHL):
                        nt = 4 * (j + 1)
                        yp = pyt.tile([128, 512], F32, tag="yt", name="yp")
                        lp = pls.tile([128, 512], F32, tag="ls", name="lp",
                                      bufs=2)
                        for t in range(nt):
                            r0 = 0 if t <= 4 * j else 128 * (t - 4 * j)
                            sp = psc.tile([128, 512], F32, tag="sc", name="sp")
                            nc.tensor.matmul(
                                sp[:, r0:512],
                                kT_sb[h][:, 128 * t:128 * (t + 1)],
                                qT_sb[h][:, 512 * j + r0:512 * (j + 1)],
                                start=True, stop=True)
                            if t >= 4 * j:
                                nc.vector.tensor_add(
                                    sp[:, r0:r0 + 128], sp[:, r0:r0 + 128],
                                    mask_sb[:])
                            pt = ptp.tile([128, 512], BF16, tag="pt", name="pt")
                            nc.scalar.activation(
                                pt[:, r0:512], sp[:, r0:512],
                                mybir.ActivationFunctionType.Exp, scale=SCALE)
                            nc.tensor.matmul(
                                lp[:, r0:512], onesb_sb[:], pt[:, r0:512],
                                start=(t == 0), stop=(t == nt - 1))
                            nc.tensor.matmul(
                                yp[:, r0:512], v_sb[t][:, 128 * h:128 * (h + 1)],
                                pt[:, r0:512],
                                start=(t == 0), stop=(t == nt - 1))
                        bcs = stp.tile([128, 512], F32, tag="bcs", name="bcs")
                        nc.vector.reciprocal(bcs[:], lp[:])
                        nc.vector.tensor_mul(yT_sb[h][j][:], yp[:], bcs[:])

            # ---------------- Phase 3: output projection ----------------
            with (
                tc.tile_pool(name="osb", bufs=2) as osb,
                tc.tile_pool(name="pout", bufs=4, space="PSUM") as pout,
            ):
                for q in range(NQ):
                    j, q4 = divmod(q, 4)
                    ot = osb.tile([128, H], F32, tag="o", name="ot")
                    pss = [pout.tile([128, 512], F32, tag="po", name="po")
                           for _ in range(4)]
                    for k in range(4):
                        for o in range(4):
                            nc.tensor.matmul(
                                pss[o][:],
                                yT_sb[k][j][:, 128 * q4:128 * (q4 + 1)],
                                wp_sb[k][:, 512 * o:512 * (o + 1)],
                                start=(k == 0), stop=(k == 3))
                    for o in range(4):
                        nc.vector.tensor_copy(
                            ot[:, 512 * o:512 * (o + 1)], pss[o][:])
                    nc.sync.dma_start(out_d[128 * q:128 * (q + 1), :], ot[:])

    nc.compile()
    return nc


def _get_nc():
    global _CACHED_NC
    if _CACHED_NC is None:
        _CACHED_NC = build_graph()
    return _CACHED_NC


def kernel(x, w_attn, w_proj):
    global LAST_RESULT
    nc = _get_nc()
    mask = np.tril(np.full((128, 128), NEG, np.float32), k=-1)
    onesb = np.ones((128, 128), bfloat16)
    in_maps = []
    for c in range(N_CORES):
        b, g = divmod(c, 4)
        lo, hi = HL * g, HL * (g + 1)
        in_maps.append({
            "xT": np.ascontiguousarray(x[b].T).astype(bfloat16),
            "wqT": np.ascontiguousarray(w_attn[lo:hi, :].T).astype(bfloat16),
            "wkT": np.ascontiguousarray(w_attn[H + lo:H + hi, :].T).astype(bfloat16),
            "wvT": np.ascontiguousarray(w_attn[2 * H + lo:2 * H + hi, :].T).astype(bfloat16),
            "wpT": np.ascontiguousarray(w_proj[:, lo:hi].T).astype(bfloat16),
            "mask": mask,
            "onesb": onesb,
        })
    res = run_bass_kernel_spmd(nc, in_maps, core_ids=list(range(N_CORES)))
    LAST_RESULT = res
    outs = [res.results[c]["out"] for c in range(N_CORES)]
    out = np.empty((B, S, H), np.float32)
    out[0] = outs[0] + outs[1] + outs[2] + outs[3]
    out[1] = outs[4] + outs[5] + outs[6] + outs[7]
    return out


# revision 14
# speedup vs baseline: 1.1616x; 1.1616x over previous
"""Distributed causal multi-head attention (B=2, S=2048, H=2048, 16 heads) on
8 TRN2 NeuronCores.

Sharding: core c handles batch b = c // 4 and the 4-head group g = c % 4
(heads 4g..4g+3). Each core computes its heads' QKV projection, causal
attention, and the partial output projection against its 512 columns of
w_proj (Megatron row-parallel). No on-device collectives: the 4 partial
outputs per batch are summed on the host during unsharding.

Device compute is bf16 on the TensorEngine with fp32 PSUM accumulation;
softmax runs in fp32. Scores are bounded (~±5 post-scale) for this data
distribution, so exp needs no max-subtraction (softmax is shift-invariant,
no overflow risk). Attention scores are computed directly in transposed
orientation (scoresT[k_pos, q_pos] via lhsT=k-tile, rhs=q-block), which is
what the PV matmul wants as its moving operand — no on-chip transposes at
all. The softmax denominator comes from a ones matmul over probsT (column
sums, replicated to all 128 partitions so the reciprocal runs wide), and
1/l is applied when draining the PV accumulator. The v-projection is
interleaved into the attention loop so attention starts right after the
q/k projections.
"""
import sys

for _p in ("/opt/trn_rl_repo", "/opt/pypackages"):
    if _p not in sys.path:
        sys.path.append(_p)

import numpy as np
from ml_dtypes import bfloat16

import concourse.bass as bass  # noqa: F401
import concourse.mybir as mybir
import concourse.tile as tile
import concourse.bacc as bacc
from concourse.bass_utils import run_bass_kernel_spmd

B, S, H = 2, 2048, 2048
NH = 16
HD = 128
NHL = 4                      # heads per core
HL = NHL * HD                # 512 local head-dims
N_CORES = 8
F32 = mybir.dt.float32
BF16 = mybir.dt.bfloat16
SCALE = float(1.0 / np.sqrt(HD))
NEG = -679.0                 # additive pre-scale mask; ~ -60 after SCALE

NHT = H // 128               # 16 contraction tiles
NQ = S // 128                # 16 query sub-tiles
NBLK = 4                     # 512-wide query blocks

LAST_RESULT = None
_CACHED_NC = None


def build_graph():
    nc = bacc.Bacc("TRN2", target_bir_lowering=False, num_devices=N_CORES)
    xT_d = nc.declare_dram_parameter("xT", [H, S], BF16, isOutput=False)
    wqT_d = nc.declare_dram_parameter("wqT", [H, HL], BF16, isOutput=False)
    wkT_d = nc.declare_dram_parameter("wkT", [H, HL], BF16, isOutput=False)
    wvT_d = nc.declare_dram_parameter("wvT", [H, HL], BF16, isOutput=False)
    wpT_d = nc.declare_dram_parameter("wpT", [HL, H], BF16, isOutput=False)
    # strict-lower-triangular NEG in [k_pos, q_pos] orientation
    mask_d = nc.declare_dram_parameter("mask", [128, 128], F32, isOutput=False)
    onesb_d = nc.declare_dram_parameter("onesb", [128, 128], BF16, isOutput=False)
    out_d = nc.declare_dram_parameter("out", [S, H], F32, isOutput=True)

    with tile.TileContext(nc) as tc:
        with tc.tile_pool(name="persist", bufs=1) as pp:
            mask_sb = pp.tile([128, 128], F32, tag="mask", name="mask_sb")
            nc.sync.dma_start(mask_sb[:], mask_d[:])
            onesb_sb = pp.tile([128, 128], BF16, tag="onesb", name="onesb_sb")
            nc.sync.dma_start(onesb_sb[:], onesb_d[:])
            wp_sb = [pp.tile([128, H], BF16, tag=f"wp{k}", name=f"wp{k}")
                     for k in range(4)]
            qT_sb = [pp.tile([128, S], BF16, tag=f"q{h}", name=f"qT{h}")
                     for h in range(NHL)]
            kT_sb = [pp.tile([128, S], BF16, tag=f"k{h}", name=f"kT{h}")
                     for h in range(NHL)]
            v_sb = [pp.tile([128, HL], BF16, tag=f"v{t}", name=f"v{t}")
                    for t in range(NQ)]
            wv_sb = [pp.tile([128, HL], BF16, tag=f"wv{h}", name=f"wvs{h}")
                     for h in range(NHT)]
            yT_sb = [[pp.tile([128, 512], BF16, tag=f"y{h}_{j}", name=f"y{h}_{j}")
                      for j in range(NBLK)] for h in range(NHL)]
            xT_sb = [pp.tile([128, S], BF16, tag=f"x{h}", name=f"x{h}")
                     for h in range(NHT)]
            x_loaded = [False] * NHT

            def load_x(h):
                if not x_loaded[h]:
                    nc.sync.dma_start(
                        xT_sb[h][:], xT_d[128 * h:128 * (h + 1), :])
                    x_loaded[h] = True

            # ---------------- Phase 1: Q/K projections ----------------
            with (
                tc.tile_pool(name="wstream", bufs=6) as wsp,
                tc.tile_pool(name="pmm", bufs=4, space="PSUM") as pmm,
            ):
                def proj_qk(wdram, dst):
                    for head in range(NHL):
                        pss = [pmm.tile([128, 512], F32, tag="mm", name="mm")
                               for _ in range(4)]
                        for h in range(NHT):
                            load_x(h)
                            wt = wsp.tile([128, 128], BF16, tag="w", name="w")
                            nc.sync.dma_start(
                                wt[:],
                                wdram[128 * h:128 * (h + 1),
                                      128 * head:128 * (head + 1)])
                            for s in range(4):
                                nc.tensor.matmul(
                                    pss[s][:], wt[:],
                                    xT_sb[h][:, 512 * s:512 * (s + 1)],
                                    start=(h == 0), stop=(h == NHT - 1))
                        for s in range(4):
                            nc.vector.tensor_copy(
                                dst[head][:, 512 * s:512 * (s + 1)], pss[s][:])

                proj_qk(wqT_d, qT_sb)
                for k in range(4):
                    nc.sync.dma_start(wp_sb[k][:],
                                      wpT_d[128 * k:128 * (k + 1), :])
                proj_qk(wkT_d, kT_sb)
                for h in range(NHT):
                    nc.sync.dma_start(wv_sb[h][:],
                                      wvT_d[128 * h:128 * (h + 1), :])

            # ---------- Phase 2: v-projection + causal attention ----------
            with (
                tc.tile_pool(name="ptp", bufs=4) as ptp,
                tc.tile_pool(name="stat", bufs=6) as stp,
                tc.tile_pool(name="psc", bufs=4, space="PSUM") as psc,
                tc.tile_pool(name="pyt", bufs=2, space="PSUM") as pyt,
                tc.tile_pool(name="pls", bufs=1, space="PSUM") as pls,
            ):
                for j in range(NBLK):
                    for st in range(4 * j, 4 * j + 4):
                        ps = psc.tile([128, 512], F32, tag="sc", name="vmm")
                        for h2 in range(NHT):
                            nc.tensor.matmul(
                                ps[:], xT_sb[h2][:, 128 * st:128 * (st + 1)],
                                wv_sb[h2][:], start=(h2 == 0),
                                stop=(h2 == NHT - 1))
                        nc.vector.tensor_copy(v_sb[st][:], ps[:])
                    for h in range(NHL):
                        nt = 4 * (j + 1)
                        yp = pyt.tile([128, 512], F32, tag="yt", name="yp")
                        lp = pls.tile([128, 512], F32, tag="ls", name="lp",
                                      bufs=2)
                        for t in range(nt):
                            r0 = 0 if t <= 4 * j else 128 * (t - 4 * j)
                            sp = psc.tile([128, 512], F32, tag="sc", name="sp")
                            nc.tensor.matmul(
                                sp[:, r0:512],
                                kT_sb[h][:, 128 * t:128 * (t + 1)],
                                qT_sb[h][:, 512 * j + r0:512 * (j + 1)],
                                start=True, stop=True)
                            if t >= 4 * j:
                                nc.vector.tensor_add(
                                    sp[:, r0:r0 + 128], sp[:, r0:r0 + 128],
                                    mask_sb[:])
                            pt = ptp.tile([128, 512], BF16, tag="pt", name="pt")
                            nc.scalar.activation(
                                pt[:, r0:512], sp[:, r0:512],
                                mybir.ActivationFunctionType.Exp, scale=SCALE)
                            nc.tensor.matmul(
                                lp[:, r0:512], onesb_sb[:], pt[:, r0:512],
                                start=(t == 0), stop=(t == nt - 1))
                            nc.tensor.matmul(
                                yp[:, r0:512], v_sb[t][:, 128 * h:128 * (h + 1)],
                                pt[:, r0:512],
                                start=(t == 0), stop=(t == nt - 1))
                        bcs = stp.tile([128, 512], F32, tag="bcs", name="bcs")
                        nc.vector.reciprocal(bcs[:], lp[:])
                        nc.vector.tensor_mul(yT_sb[h][j][:], yp[:], bcs[:])

            # ---------------- Phase 3: output projection ----------------
            with (
                tc.tile_pool(name="osb", bufs=2) as osb,
                tc.tile_pool(name="pout", bufs=4, space="PSUM") as pout,
            ):
                for q in range(NQ):
                    j, q4 = divmod(q, 4)
                    ot = osb.tile([128, H], F32, tag="o", name="ot")
                    pss = [pout.tile([128, 512], F32, tag="po", name="po")
                           for _ in range(4)]
                    for k in range(4):
                        for o in range(4):
                            nc.tensor.matmul(
                                pss[o][:],
                                yT_sb[k][j][:, 128 * q4:128 * (q4 + 1)],
                                wp_sb[k][:, 512 * o:512 * (o + 1)],
                                start=(k == 0), stop=(k == 3))
                    for o in range(4):
                        nc.vector.tensor_copy(
                            ot[:, 512 * o:512 * (o + 1)], pss[o][:])
                    nc.sync.dma_start(out_d[128 * q:128 * (q + 1), :], ot[:])

    nc.compile()
    return nc


def _get_nc():
    global _CACHED_NC
    if _CACHED_NC is None:
        _CACHED_NC = build_graph()
    return _CACHED_NC


def kernel(x, w_attn, w_proj):
    global LAST_RESULT
    nc = _get_nc()
    mask = np.tril(np.full((128, 128), NEG, np.float32), k=-1)
    onesb = np.ones((128, 128), bfloat16)
    in_maps = []
    for c in range(N_CORES):
        b, g = divmod(c, 4)
        lo, hi = HL * g, HL * (g + 1)
        in_maps.append({
            "xT": np.ascontiguousarray(x[b].T).astype(bfloat16),
            "wqT": np.ascontiguousarray(w_attn[lo:hi, :].T).astype(bfloat16),
            "wkT": np.ascontiguousarray(w_attn[H + lo:H + hi, :].T).astype(bfloat16),
            "wvT": np.ascontiguousarray(w_attn[2 * H + lo:2 * H + hi, :].T).astype(bfloat16),
            "wpT": np.ascontiguousarray(w_proj[:, lo:hi].T).astype(bfloat16),
            "mask": mask,
            "onesb": onesb,
        })
    res = run_bass_kernel_spmd(nc, in_maps, core_ids=list(range(N_CORES)))
    LAST_RESULT = res
    outs = [res.results[c]["out"] for c in range(N_CORES)]
    out = np.empty((B, S, H), np.float32)
    out[0] = outs[0] + outs[1] + outs[2] + outs[3]
    out[1] = outs[4] + outs[5] + outs[6] + outs[7]
    return out
